# revision 7
# baseline (speedup 1.0000x reference)
"""Bass/Trainium2 kernel for nn_LocalSingularityStrength.

Reference computation (per sample):
  xs = (x - mn) / (mx - mn + EPS)            # min/max over whole sample
  m_r = boxsum_rxr(xs), r in [2,4,8,16]      # SAME padding
  alphas = sum_r w_r * ln(m_r + EPS)         # OLS slope of ln m vs ln r
  out = (alphas - mean) * rsqrt(var+BN_EPS) * gamma + beta

Key algebra used here:
  * sum_r w_r = 0  =>  the 1/(mx-mn+EPS) scale cancels: with B_r = boxsum_r(x-mn),
    alphas = sum_r w_r * ln(B_r + EPS')  where EPS' = EPS*(mx-mn+EPS).  Exact.
  * BN folds to per-channel affine out = alphas*G + Bc; for the benchmarked
    inputs G/Bc are channel-uniform, folded into scalar immediates g, b.
  * W-axis box sums via a doubling chain of shifted adds (every scale is an
    intermediate); H-axis box sums + scale-combine via TensorE banded/diagonal
    matmuls with fp32 PSUM accumulation.

Sharding: pure data parallel, 2 samples per core across 8 cores.
"""

import math
import numpy as np

B, H, W, C = 16, 224, 224, 32
N_CORES = 8
BPC = B // N_CORES            # samples per core
EPS = 1e-7
BN_EPS = 1e-3
SCALES = [16, 8, 4, 2]        # processing order (16 first: frees PSUM early)
PADLO = {2: 0, 4: 1, 8: 3, 16: 7}   # SAME padding, left/top pad per scale
HT = 112                      # output rows per H-tile
HALO_T, HALO_B = 7, 8         # halo rows above/below (for r=16 window)
KROWS = HT + HALO_T + HALO_B  # 127 input rows per tile
WM = 8                        # W margin (columns) on each side
WP = (W + 2 * WM) * C         # padded free size = 7680
FD = W * C                    # data free size = 7168
NCHUNK = 512                  # free-dim chunk for matmul/log/combine stages
NCH = FD // NCHUNK            # 14 chunks per tile
SR = {2: 0.5, 4: 0.125, 8: 0.03125, 16: 0.0078125}  # log-centering prescale

_CACHE = {}


def _weights():
    ls = np.log(np.array([2.0, 4.0, 8.0, 16.0], np.float64))
    lc = ls - ls.mean()
    return lc / (lc * lc).sum()          # w for scales [2,4,8,16]


def _host_consts(gamma, beta, moving_mean, moving_var):
    g64 = gamma.astype(np.float64)
    inv = 1.0 / np.sqrt(moving_var.astype(np.float64) + BN_EPS)
    G = g64 * inv
    Bc = beta.astype(np.float64) - moving_mean.astype(np.float64) * G
    uni = (np.ptp(G) <= 1e-12 * max(1.0, abs(G[0]))) and (
        np.ptp(Bc) <= 1e-12 * max(1.0, abs(Bc[0])))
    w = _weights()                        # [w2, w4, w8, w16]
    wmap = {2: w[0], 4: w[1], 8: w[2], 16: w[3]}
    g = float(G[0]) if uni else 1.0
    b = float(Bc[0]) if uni else 0.0
    # K corrects for the ln prescale s_r:  u = sum c_r ln(s_r (m+eps'))
    K = -sum(g * wmap[r] * math.log(SR[r]) for r in SCALES)
    b_total = b + K

    # Banded H-window matrices, [KROWS, HT], one per tile. Tile t loads H
    # rows [row_base, row_base+127) at partitions 0..126; SAME padding is
    # realized by clipping the band to valid rows.
    bands = np.zeros((2, len(SCALES), KROWS, HT), np.float32)
    for t, row_base in enumerate((0, H - KROWS)):
        for si, r in enumerate(SCALES):
            pb = PADLO[r]
            for o in range(HT):
                h = t * HT + o
                for row in range(h - pb, h - pb + r):
                    k = row - row_base
                    if 0 <= row < H and 0 <= k < KROWS:
                        bands[t, si, k, o] = 1.0
    # Diagonal combine matrices c_r * I, [HT, HT].
    diags = np.zeros((len(SCALES), HT, HT), np.float32)
    for si, r in enumerate(SCALES):
        np.fill_diagonal(diags[si], g * wmap[r])
    params = np.array([b_total, 0.0], np.float32)
    return (bands.astype(np.float16), diags.astype(np.float16), params,
            uni, G.astype(np.float32), Bc.astype(np.float32), K)


def _build_nc():
    if "nc" in _CACHE:
        return _CACHE["nc"]
    import concourse.bass as bass
    import concourse.tile as tile
    from concourse import mybir, bacc, bass_isa
    from contextlib import ExitStack

    f32, f16 = mybir.dt.float32, mybir.dt.float16
    ALU = mybir.AluOpType
    AF = mybir.ActivationFunctionType

    nc = bacc.Bacc("TRN2", target_bir_lowering=False, debug=False,
                   num_devices=N_CORES)
    x_d = nc.dram_tensor("xs", [BPC, H, W, C], f32, kind="ExternalInput").ap()
    bands_d = nc.dram_tensor("bands", [2, 4, KROWS, HT], f16,
                             kind="ExternalInput").ap()
    diags_d = nc.dram_tensor("diags", [4, HT, HT], f16,
                             kind="ExternalInput").ap()
    params_d = nc.dram_tensor("params", [2], f32, kind="ExternalInput").ap()
    out_d = nc.dram_tensor("out", [BPC, H, W, C], f32,
                           kind="ExternalOutput").ap()

    with tile.TileContext(nc) as tc, ExitStack() as ctx:
        P = lambda name, bufs, **kw: ctx.enter_context(
            tc.tile_pool(name=name, bufs=bufs, **kw))
        singles = P("singles", 1)
        xpool = P("xpool", 2)
        xhpool = P("xhpool", 2)
        spool = P("spool", 1)
        lmpool = P("lmpool", 3)
        outpool = P("outpool", 4)
        scal = P("scal", 2)
        ps_m16 = P("ps_m16", 2, space="PSUM")
        ps_m8 = P("ps_m8", 1, space="PSUM")
        ps_m4 = P("ps_m4", 1, space="PSUM")
        ps_m2 = P("ps_m2", 2, space="PSUM")
        ps_u = P("ps_u", 2, space="PSUM")
        ps_pool = {16: ps_m16, 8: ps_m8, 4: ps_m4, 2: ps_m2}

        # --- constants to SBUF ---
        bands_sb = [singles.tile([KROWS, 4, HT], f16, tag=f"bands{t}",
                                 name=f"bands_sb{t}") for t in range(2)]
        for t in range(2):
            nc.sync.dma_start(bands_sb[t][:],
                              bands_d[t].transpose([1, 0, 2]))
        diags_sb = singles.tile([HT, 4, HT], f16, tag="diags")
        nc.sync.dma_start(diags_sb[:], diags_d.transpose([1, 0, 2]))
        btot = singles.tile([128, 1], f32, tag="btot")
        nc.sync.dma_start(
            btot[:], bass.AP(tensor=params_d.tensor, offset=0,
                             ap=[[0, 128], [1, 1]]))

        # persistent fp16 tiles: memset once so stale data is always finite
        xh_bufs = [singles.tile([KROWS, WP], f16, tag=f"xh{i}", name=f"xh{i}")
                   for i in range(2)]
        for t in xh_bufs:
            nc.vector.memset(t[:], 0.0)

        # per-tile DRAM H-row base: both tiles load 127 real rows at part 0
        tbase = (0, H - KROWS)

        prev = None   # pending (lm tiles dict, u-slot chunk info) for skew

        def flush_prev():
            nonlocal prev
            if prev is None:
                return
            lmti, s_, t_, c_ = prev
            u = ps_u.tile([HT, NCHUNK], f32, tag="u")
            for i, r in enumerate(SCALES):
                nc.tensor.matmul(u[:], diags_sb[:, i, :], lmti[r][:],
                                 start=(i == 0), stop=(i == 3))
            osb = outpool.tile([HT, NCHUNK], f32, tag="osb")
            if c_ % 4 == 3:   # every 4th chunk on ACT to offload DVE a bit
                nc.scalar.activation(osb[:], u[:], AF.Identity,
                                     bias=btot[0:HT], scale=1.0)
            else:
                nc.vector.tensor_scalar_add(osb[:], u[:], btot[0:HT])
            w0 = c_ * (NCHUNK // C)
            nc.sync.dma_start(
                out_d[s_, t_ * HT:(t_ + 1) * HT, w0:w0 + NCHUNK // C, :],
                osb[:])
            prev = None

        for s in range(BPC):
            # ---- load + cast(+min) + max partials ----
            mn_strip = scal.tile([128, 2], f32, tag="mnst")
            mx_strip = scal.tile([128, 2], f32, tag="mxst")
            nc.vector.memset(mn_strip[:], 3.0e38)
            nc.vector.memset(mx_strip[:], -3.0e38)
            xhs = []
            for t in range(2):
                h0 = tbase[t]
                xt = xpool.tile([KROWS, FD], f32, tag="x")
                nc.sync.dma_start(xt[:, :], x_d[s, h0:h0 + KROWS, :, :])
                xh = xh_bufs[t]
                # cast f32->f16 (tensor_tensor_reduce would fuse the min here
                # but wedges the device, so cast + reduce separately)
                nc.vector.tensor_copy(xh[:, WM * C:WM * C + FD], xt[:, :])
                nc.vector.tensor_reduce(
                    out=mn_strip[0:KROWS, t:t + 1],
                    in_=xh[:, WM * C:WM * C + FD],
                    axis=mybir.AxisListType.X, op=ALU.min)
                # subsampled max partials (enters only via tiny EPS')
                xv = xh[:, WM * C:WM * C + FD].rearrange(
                    "p (w c) -> p w c", c=C)[:, ::4, :]
                nc.vector.tensor_reduce(
                    out=mx_strip[0:KROWS, t:t + 1], in_=xv,
                    axis=mybir.AxisListType.XY, op=ALU.max)
                xhs.append(xh)

            # ---- finalize mn / mx / eps' (tiny ops) ----
            mncol = scal.tile([128, 1], f32, tag="mncol")
            nc.vector.tensor_tensor(mncol[:], mn_strip[:, 0:1],
                                    mn_strip[:, 1:2], op=ALU.min)
            nc.vector.tensor_scalar_mul(mncol[:], mncol[:], -1.0)
            mn_bc = scal.tile([128, 1], f32, tag="mnbc")
            nc.gpsimd.partition_all_reduce(mn_bc[:], mncol[:], channels=128,
                                           reduce_op=bass_isa.ReduceOp.max)
            nc.vector.tensor_scalar_mul(mn_bc[:], mn_bc[:], -1.0)
            mxcol = scal.tile([128, 1], f32, tag="mxcol")
            nc.vector.tensor_tensor(mxcol[:], mx_strip[:, 0:1],
                                    mx_strip[:, 1:2], op=ALU.max)
            mx_bc = scal.tile([128, 1], f32, tag="mxbc")
            nc.gpsimd.partition_all_reduce(mx_bc[:], mxcol[:], channels=128,
                                           reduce_op=bass_isa.ReduceOp.max)
            m2n = scal.tile([128, 1], f32, tag="m2n")
            nc.vector.tensor_scalar_mul(m2n[:], mn_bc[:], 2.0)
            epsp = scal.tile([128, 1], f32, tag="epsp")
            nc.vector.tensor_tensor(epsp[:], mx_bc[:], mn_bc[:],
                                    op=ALU.subtract)
            nc.vector.tensor_scalar(epsp[:], epsp[:], EPS, EPS,
                                    op0=ALU.add, op1=ALU.mult)
            epsb = {}
            for r in SCALES:
                e = scal.tile([128, 1], f32, tag=f"epsb{r}", name=f"epsb{r}")
                nc.vector.tensor_scalar_mul(e[:], epsp[:], SR[r])
                epsb[r] = e

            for t in range(2):
                xh = xhs[t]
                # fill W margins with mn: after x-mn they contribute zero
                for lo, hi in ((0, WM * C), (WM * C + FD, WP)):
                    nc.scalar.activation(xh[:, lo:hi], xh[:, lo:hi],
                                         AF.Identity, bias=mn_bc[0:KROWS],
                                         scale=0.0)
                # ---- W-axis doubling chain (all 127 rows; halo rows are
                # garbage but get multiplied by 0 in the banded lhsT) ----
                S = {}
                S[2] = spool.tile([KROWS, WP], f16, tag="S2", name="S2t")
                nc.vector.scalar_tensor_tensor(
                    out=S[2][:, 32:7648], in0=xh[:, 32:7648],
                    scalar=m2n[0:KROWS], in1=xh[:, 64:7680],
                    op0=ALU.subtract, op1=ALU.add)
                S[4] = spool.tile([KROWS, WP], f16, tag="S4", name="S4t")
                nc.vector.tensor_tensor(S[4][:, 64:7616], S[2][:, 32:7584],
                                        S[2][:, 96:7648], op=ALU.add)
                S[8] = spool.tile([KROWS, WP], f16, tag="S8", name="S8t")
                nc.vector.tensor_tensor(S[8][:, 128:7552], S[4][:, 64:7488],
                                        S[4][:, 192:7616], op=ALU.add)
                S[16] = spool.tile([KROWS, WP], f16, tag="S16", name="S16t")
                nc.vector.tensor_tensor(S[16][:, 256:7424], S[8][:, 128:7296],
                                        S[8][:, 384:7552], op=ALU.add)

                # ---- per-chunk: H matmuls -> ln -> (skewed) combine ----
                for c in range(NCH):
                    fo = WM * C + c * NCHUNK
                    mt = {}
                    for si, r in enumerate(SCALES):
                        m = ps_pool[r].tile([HT, NCHUNK], f32, tag=f"m{r}", name=f"mps{r}")
                        nc.tensor.matmul(m[:], bands_sb[t][:, si, :],
                                         S[r][:, fo:fo + NCHUNK],
                                         start=True, stop=True)
                        mt[r] = m
                    flush_prev()
                    lmt = {}
                    for r in SCALES:
                        lm = lmpool.tile([HT, NCHUNK], f16, tag=f"lm{r}", name=f"lm{r}")
                        nc.scalar.activation(lm[:], mt[r][:], AF.Ln,
                                             bias=epsb[r][0:HT], scale=SR[r])
                        lmt[r] = lm
                    prev = (lmt, s, t, c)
        flush_prev()
    nc.compile()
    _CACHE["nc"] = nc
    return nc


def kernel(x, gamma, beta, moving_mean, moving_var):
    from concourse.bass_utils import run_bass_kernel_spmd

    x = np.ascontiguousarray(np.asarray(x, np.float32))
    bands, diags, params, uni, G, Bc, K = _host_consts(
        np.asarray(gamma), np.asarray(beta),
        np.asarray(moving_mean), np.asarray(moving_var))
    nc = _build_nc()
    in_maps = [{"xs": x[c * BPC:(c + 1) * BPC], "bands": bands,
                "diags": diags, "params": params} for c in range(N_CORES)]
    res = run_bass_kernel_spmd(nc, in_maps, core_ids=list(range(N_CORES)))
    out = np.concatenate([res.results[c]["out"] for c in range(N_CORES)],
                         axis=0)
    if not uni:
        # general fallback: device ran with g=1,b=0 => out holds raw alphas
        out = out * G[None, None, None, :] + Bc[None, None, None, :]
    return out.astype(np.float32)


# revision 14
# speedup vs baseline: 1.1202x; 1.1202x over previous
"""Bass/Trainium2 kernel for nn_LocalSingularityStrength.

Reference computation (per sample):
  xs = (x - mn) / (mx - mn + EPS)            # min/max over whole sample
  m_r = boxsum_rxr(xs), r in [2,4,8,16]      # SAME padding
  alphas = sum_r w_r * ln(m_r + EPS)         # OLS slope of ln m vs ln r
  out = (alphas - mean) * rsqrt(var+BN_EPS) * gamma + beta

Key algebra used here:
  * sum_r w_r = 0  =>  the 1/(mx-mn+EPS) scale cancels: with B_r = boxsum_r(x-mn),
    alphas = sum_r w_r * ln(B_r + EPS')  where EPS' = EPS*(mx-mn+EPS).  Exact.
  * BN folds to per-channel affine out = alphas*G + Bc; for the benchmarked
    inputs G/Bc are channel-uniform, folded into scalar immediates g, b.
  * W-axis box sums via a doubling chain of shifted adds (every scale is an
    intermediate); H-axis box sums + scale-combine via TensorE banded/diagonal
    matmuls with fp32 PSUM accumulation.

Sharding: pure data parallel, 2 samples per core across 8 cores.
"""

import math
import numpy as np

B, H, W, C = 16, 224, 224, 32
N_CORES = 8
BPC = B // N_CORES            # samples per core
EPS = 1e-7
BN_EPS = 1e-3
SCALES = [16, 8, 4, 2]        # processing order (16 first: frees PSUM early)
PADLO = {2: 0, 4: 1, 8: 3, 16: 7}   # SAME padding, left/top pad per scale
HT = 112                      # output rows per H-tile
HALO_T, HALO_B = 7, 8         # halo rows above/below (for r=16 window)
KROWS = HT + HALO_T + HALO_B  # 127 input rows per tile
WM = 8                        # W margin (columns) on each side
WP = (W + 2 * WM) * C         # padded free size = 7680
FD = W * C                    # data free size = 7168
NCHUNK = 512                  # free-dim chunk for matmul/log/combine stages
NCH = FD // NCHUNK            # 14 chunks per tile
# log-centering prescale, shared within PSUM pair-groups (16,8) and (4,2)
SR = {16: 1.0 / 64, 8: 1.0 / 64, 4: 0.25, 2: 0.25}

_CACHE = {}


def _weights():
    ls = np.log(np.array([2.0, 4.0, 8.0, 16.0], np.float64))
    lc = ls - ls.mean()
    return lc / (lc * lc).sum()          # w for scales [2,4,8,16]


def _host_consts(gamma, beta, moving_mean, moving_var):
    g64 = gamma.astype(np.float64)
    inv = 1.0 / np.sqrt(moving_var.astype(np.float64) + BN_EPS)
    G = g64 * inv
    Bc = beta.astype(np.float64) - moving_mean.astype(np.float64) * G
    uni = (np.ptp(G) <= 1e-12 * max(1.0, abs(G[0]))) and (
        np.ptp(Bc) <= 1e-12 * max(1.0, abs(Bc[0])))
    w = _weights()                        # [w2, w4, w8, w16]
    wmap = {2: w[0], 4: w[1], 8: w[2], 16: w[3]}
    g = float(G[0]) if uni else 1.0
    b = float(Bc[0]) if uni else 0.0
    # K corrects for the ln prescale s_r:  u = sum c_r ln(s_r (m+eps'))
    K = -sum(g * wmap[r] * math.log(SR[r]) for r in SCALES)
    b_total = b + K

    # Banded H-window matrices, [KROWS, HT], one per tile. Tile t loads H
    # rows [row_base, row_base+127) at partitions 0..126; SAME padding is
    # realized by clipping the band to valid rows.
    bands = np.zeros((2, len(SCALES), KROWS, HT), np.float32)
    for t, row_base in enumerate((0, H - KROWS)):
        for si, r in enumerate(SCALES):
            pb = PADLO[r]
            for o in range(HT):
                h = t * HT + o
                for row in range(h - pb, h - pb + r):
                    k = row - row_base
                    if 0 <= row < H and 0 <= k < KROWS:
                        bands[t, si, k, o] = 1.0
    # Diagonal combine matrices c_r * I, [HT, HT].
    diags = np.zeros((len(SCALES), HT, HT), np.float32)
    for si, r in enumerate(SCALES):
        np.fill_diagonal(diags[si], g * wmap[r])
    params = np.array([b_total, 0.0], np.float32)
    return (bands.astype(np.float16), diags.astype(np.float16), params,
            uni, G.astype(np.float32), Bc.astype(np.float32), K)


def _build_nc():
    if "nc" in _CACHE:
        return _CACHE["nc"]
    import concourse.bass as bass
    import concourse.tile as tile
    from concourse import mybir, bacc, bass_isa
    from contextlib import ExitStack

    f32, f16 = mybir.dt.float32, mybir.dt.float16
    ALU = mybir.AluOpType
    AF = mybir.ActivationFunctionType

    nc = bacc.Bacc("TRN2", target_bir_lowering=False, debug=False,
                   num_devices=N_CORES)
    x_d = nc.dram_tensor("xs", [BPC, H, W, C], f32, kind="ExternalInput").ap()
    bands_d = nc.dram_tensor("bands", [2, 4, KROWS, HT], f16,
                             kind="ExternalInput").ap()
    diags_d = nc.dram_tensor("diags", [4, HT, HT], f16,
                             kind="ExternalInput").ap()
    params_d = nc.dram_tensor("params", [2], f32, kind="ExternalInput").ap()
    out_d = nc.dram_tensor("out", [BPC, H, W, C], f32,
                           kind="ExternalOutput").ap()

    with tile.TileContext(nc) as tc, ExitStack() as ctx:
        P = lambda name, bufs, **kw: ctx.enter_context(
            tc.tile_pool(name=name, bufs=bufs, **kw))
        singles = P("singles", 1)
        xhpool = P("xhpool", 4)
        spool = P("spool", 1)
        lmpool = P("lmpool", 3)
        outpool = P("outpool", 4)
        scal = P("scal", 2)
        ps_A = P("ps_A", 2, space="PSUM")   # scales (16, 8): 2 banks/tile
        ps_B = P("ps_B", 1, space="PSUM")   # scales (4, 2)
        ps_u = P("ps_u", 2, space="PSUM")

        # --- constants to SBUF ---
        bands_sb = [singles.tile([KROWS, 4, HT], f16, tag=f"bands{t}",
                                 name=f"bands_sb{t}") for t in range(2)]
        for t in range(2):
            nc.sync.dma_start(bands_sb[t][:],
                              bands_d[t].transpose([1, 0, 2]))
        diags_sb = singles.tile([HT, 4, HT], f16, tag="diags")
        nc.sync.dma_start(diags_sb[:], diags_d.transpose([1, 0, 2]))
        btot = singles.tile([128, 1], f32, tag="btot")
        nc.sync.dma_start(
            btot[:], bass.AP(tensor=params_d.tensor, offset=0,
                             ap=[[0, 128], [1, 1]]))

        # per-tile DRAM H-row base: both tiles load 127 real rows at part 0
        tbase = (0, H - KROWS)

        prev = None   # pending (lm tiles dict, u-slot chunk info) for skew

        def flush_prev():
            nonlocal prev
            if prev is None:
                return
            (lmA, lmB), s_, t_, c_ = prev
            # rhs halves: lmA = [lm16 | lm8], lmB = [lm4 | lm2]
            rhs = {16: lmA[:, 0:NCHUNK], 8: lmA[:, NCHUNK:2 * NCHUNK],
                   4: lmB[:, 0:NCHUNK], 2: lmB[:, NCHUNK:2 * NCHUNK]}
            u = ps_u.tile([HT, NCHUNK], f32, tag="u")
            for i, r in enumerate(SCALES):
                nc.tensor.matmul(u[:], diags_sb[:, i, :], rhs[r],
                                 start=(i == 0), stop=(i == 3))
            osb = outpool.tile([HT, NCHUNK], f32, tag="osb")
            if c_ % 4 == 3:   # every 4th chunk on ACT to offload DVE a bit
                nc.scalar.activation(osb[:], u[:], AF.Identity,
                                     bias=btot[0:HT], scale=1.0)
            else:
                nc.vector.tensor_scalar_add(osb[:], u[:], btot[0:HT])
            w0 = c_ * (NCHUNK // C)
            nc.sync.dma_start(
                out_d[s_, t_ * HT:(t_ + 1) * HT, w0:w0 + NCHUNK // C, :],
                osb[:])
            prev = None

        for s in range(BPC):
            # ---- load + cast(+min) + max partials ----
            mn_strip = scal.tile([128, 2], f32, tag="mnst")
            mx_strip = scal.tile([128, 2], f32, tag="mxst")
            nc.vector.memset(mn_strip[:], 3.0e38)
            nc.vector.memset(mx_strip[:], -3.0e38)
            xhs = []
            for t in range(2):
                h0 = tbase[t]
                xh = xhpool.tile([KROWS, WP], f16, tag="xh", name="xh")
                # zero margins (memset is write-only: safe on garbage slots)
                nc.vector.memset(xh[:, 0:WM * C], 0.0)
                nc.vector.memset(xh[:, WM * C + FD:WP], 0.0)
                # casting DMA: SWDGE converts f32->f16 in the DMA datapath
                nc.gpsimd.dma_start(xh[:, WM * C:WM * C + FD],
                                    x_d[s, h0:h0 + KROWS, :, :])
                # min over the f16 values actually used downstream (exact
                # within the compute domain; diff vs f32 min is <= 1 ulp(mn))
                nc.vector.tensor_reduce(
                    out=mn_strip[0:KROWS, t:t + 1],
                    in_=xh[:, WM * C:WM * C + FD],
                    axis=mybir.AxisListType.X, op=ALU.min)
                # subsampled max partials (enters only via tiny EPS')
                xv = xh[:, WM * C:WM * C + FD].rearrange(
                    "p (w c) -> p w c", c=C)[:, ::8, :]
                nc.vector.tensor_reduce(
                    out=mx_strip[0:KROWS, t:t + 1], in_=xv,
                    axis=mybir.AxisListType.XY, op=ALU.max)
                xhs.append(xh)

            # ---- finalize mn / mx / eps' (tiny ops) ----
            mncol = scal.tile([128, 1], f32, tag="mncol")
            nc.vector.tensor_tensor(mncol[:], mn_strip[:, 0:1],
                                    mn_strip[:, 1:2], op=ALU.min)
            nc.vector.tensor_scalar_mul(mncol[:], mncol[:], -1.0)
            mn_bc = scal.tile([128, 1], f32, tag="mnbc")
            nc.gpsimd.partition_all_reduce(mn_bc[:], mncol[:], channels=128,
                                           reduce_op=bass_isa.ReduceOp.max)
            nc.vector.tensor_scalar_mul(mn_bc[:], mn_bc[:], -1.0)
            mxcol = scal.tile([128, 1], f32, tag="mxcol")
            nc.vector.tensor_tensor(mxcol[:], mx_strip[:, 0:1],
                                    mx_strip[:, 1:2], op=ALU.max)
            mx_bc = scal.tile([128, 1], f32, tag="mxbc")
            nc.gpsimd.partition_all_reduce(mx_bc[:], mxcol[:], channels=128,
                                           reduce_op=bass_isa.ReduceOp.max)
            m2n = scal.tile([128, 1], f32, tag="m2n")
            nc.vector.tensor_scalar_mul(m2n[:], mn_bc[:], 2.0)
            epsp = scal.tile([128, 1], f32, tag="epsp")
            nc.vector.tensor_tensor(epsp[:], mx_bc[:], mn_bc[:],
                                    op=ALU.subtract)
            nc.vector.tensor_scalar(epsp[:], epsp[:], EPS, EPS,
                                    op0=ALU.add, op1=ALU.mult)
            epsb = {}
            for gname, r0 in (("A", 16), ("B", 4)):
                e = scal.tile([128, 1], f32, tag=f"epsb{gname}",
                              name=f"epsb{gname}")
                nc.vector.tensor_scalar_mul(e[:], epsp[:], SR[r0])
                epsb[gname] = e

            for t in range(2):
                xh = xhs[t]
                # fill W margins with mn: after x-mn they contribute zero
                for lo, hi in ((0, WM * C), (WM * C + FD, WP)):
                    nc.scalar.activation(xh[:, lo:hi], xh[:, lo:hi],
                                         AF.Identity, bias=mn_bc[0:KROWS],
                                         scale=0.0)
                # ---- W-axis doubling chain (all 127 rows; halo rows are
                # garbage but get multiplied by 0 in the banded lhsT) ----
                S = {}
                S[2] = spool.tile([KROWS, WP], f16, tag="S2", name="S2t")
                nc.vector.scalar_tensor_tensor(
                    out=S[2][:, 32:7648], in0=xh[:, 32:7648],
                    scalar=m2n[0:KROWS], in1=xh[:, 64:7680],
                    op0=ALU.subtract, op1=ALU.add)
                S[4] = spool.tile([KROWS, WP], f16, tag="S4", name="S4t")
                nc.vector.tensor_tensor(S[4][:, 64:7616], S[2][:, 32:7584],
                                        S[2][:, 96:7648], op=ALU.add)
                S[8] = spool.tile([KROWS, WP], f16, tag="S8", name="S8t")
                nc.vector.tensor_tensor(S[8][:, 128:7552], S[4][:, 64:7488],
                                        S[4][:, 192:7616], op=ALU.add)
                S[16] = spool.tile([KROWS, WP], f16, tag="S16", name="S16t")
                nc.vector.tensor_tensor(S[16][:, 256:7424], S[8][:, 128:7296],
                                        S[8][:, 384:7552], op=ALU.add)

                # ---- per-chunk: H matmuls -> ln (paired) -> skewed combine
                for c in range(NCH):
                    fo = WM * C + c * NCHUNK
                    # SCALES order [16,8,4,2]; pairs share a PSUM tile so one
                    # ACT Ln op covers both banks
                    mA = ps_A.tile([HT, 2 * NCHUNK], f32, tag="mA",
                                   name="mA")
                    mB = ps_B.tile([HT, 2 * NCHUNK], f32, tag="mB",
                                   name="mB")
                    halves = {16: mA[:, 0:NCHUNK], 8: mA[:, NCHUNK:],
                              4: mB[:, 0:NCHUNK], 2: mB[:, NCHUNK:]}
                    for si, r in enumerate(SCALES):
                        nc.tensor.matmul(halves[r], bands_sb[t][:, si, :],
                                         S[r][:, fo:fo + NCHUNK],
                                         start=True, stop=True)
                    flush_prev()
                    lmA = lmpool.tile([HT, 2 * NCHUNK], f16, tag="lmA",
                                      name="lmA")
                    nc.scalar.activation(lmA[:], mA[:], AF.Ln,
                                         bias=epsb["A"][0:HT], scale=SR[16])
                    lmB = lmpool.tile([HT, 2 * NCHUNK], f16, tag="lmB",
                                      name="lmB")
                    nc.scalar.activation(lmB[:], mB[:], AF.Ln,
                                         bias=epsb["B"][0:HT], scale=SR[4])
                    prev = ((lmA, lmB), s, t, c)
        flush_prev()
    nc.compile()
    _CACHE["nc"] = nc
    return nc


def kernel(x, gamma, beta, moving_mean, moving_var):
    from concourse.bass_utils import run_bass_kernel_spmd

    x = np.ascontiguousarray(np.asarray(x, np.float32))
    bands, diags, params, uni, G, Bc, K = _host_consts(
        np.asarray(gamma), np.asarray(beta),
        np.asarray(moving_mean), np.asarray(moving_var))
    nc = _build_nc()
    in_maps = [{"xs": x[c * BPC:(c + 1) * BPC], "bands": bands,
                "diags": diags, "params": params} for c in range(N_CORES)]
    res = run_bass_kernel_spmd(nc, in_maps, core_ids=list(range(N_CORES)))
    out = np.concatenate([res.results[c]["out"] for c in range(N_CORES)],
                         axis=0)
    if not uni:
        # general fallback: device ran with g=1,b=0 => out holds raw alphas
        out = out * G[None, None, None, :] + Bc[None, None, None, :]
    return out.astype(np.float32)


# revision 15
# speedup vs baseline: 1.2474x; 1.1136x over previous
"""Bass/Trainium2 kernel for nn_LocalSingularityStrength.

Reference computation (per sample):
  xs = (x - mn) / (mx - mn + EPS)            # min/max over whole sample
  m_r = boxsum_rxr(xs), r in [2,4,8,16]      # SAME padding
  alphas = sum_r w_r * ln(m_r + EPS)         # OLS slope of ln m vs ln r
  out = (alphas - mean) * rsqrt(var+BN_EPS) * gamma + beta

Key algebra used here:
  * sum_r w_r = 0  =>  the 1/(mx-mn+EPS) scale cancels: with B_r = boxsum_r(x-mn),
    alphas = sum_r w_r * ln(B_r + EPS')  where EPS' = EPS*(mx-mn+EPS).  Exact.
  * BN folds to per-channel affine out = alphas*G + Bc; for the benchmarked
    inputs G/Bc are channel-uniform, folded into scalar immediates g, b.
  * W-axis box sums via a doubling chain of shifted adds (every scale is an
    intermediate); H-axis box sums + scale-combine via TensorE banded/diagonal
    matmuls with fp32 PSUM accumulation; ln on ScalarE reading PSUM pairs.

Sharding: pure data parallel, 2 samples per core across 8 cores.  The
emission is software-pipelined: the next sample's casting-DMA/min/max and the
next tile's W-chain are emitted inside the current tile's chunk loop so DVE
work overlaps PE/ACT work.
"""

import math
import numpy as np

B, H, W, C = 16, 224, 224, 32
N_CORES = 8
BPC = B // N_CORES            # samples per core
EPS = 1e-7
BN_EPS = 1e-3
SCALES = [16, 8, 4, 2]        # processing order
PADLO = {2: 0, 4: 1, 8: 3, 16: 7}   # SAME padding, left/top pad per scale
HT = 112                      # output rows per H-tile
KROWS = 127                   # input rows per tile (112 + 15 window overlap)
WM = 8                        # W margin (columns) on each side
WP = (W + 2 * WM) * C         # padded free size = 7680
FD = W * C                    # data free size = 7168
NCHUNK = 512                  # free-dim chunk for matmul/log/combine stages
NCH = FD // NCHUNK            # 14 chunks per tile
# log-centering prescale, shared within PSUM pair-groups (16,8) and (4,2)
SR = {16: 1.0 / 64, 8: 1.0 / 64, 4: 0.25, 2: 0.25}
# W-chain valid ranges (element offsets into the padded free dim), from
# S2 on w in [-7,231), S4 [-6,230), S8 [-4,228), S16 [0,224):
CH_RANGE = {2: (32, 7648), 4: (64, 7616), 8: (128, 7552), 16: (256, 7424)}

_CACHE = {}


def _weights():
    ls = np.log(np.array([2.0, 4.0, 8.0, 16.0], np.float64))
    lc = ls - ls.mean()
    return lc / (lc * lc).sum()          # w for scales [2,4,8,16]


def _host_consts(gamma, beta, moving_mean, moving_var):
    g64 = gamma.astype(np.float64)
    inv = 1.0 / np.sqrt(moving_var.astype(np.float64) + BN_EPS)
    G = g64 * inv
    Bc = beta.astype(np.float64) - moving_mean.astype(np.float64) * G
    uni = (np.ptp(G) <= 1e-12 * max(1.0, abs(G[0]))) and (
        np.ptp(Bc) <= 1e-12 * max(1.0, abs(Bc[0])))
    w = _weights()                        # [w2, w4, w8, w16]
    wmap = {2: w[0], 4: w[1], 8: w[2], 16: w[3]}
    g = float(G[0]) if uni else 1.0
    b = float(Bc[0]) if uni else 0.0
    # K corrects for the ln prescale s_r:  u = sum c_r ln(s_r (m+eps'))
    K = -sum(g * wmap[r] * math.log(SR[r]) for r in SCALES)
    b_total = b + K

    # Banded H-window matrices, [KROWS, HT], one per tile. Tile t loads H
    # rows [row_base, row_base+127) at partitions 0..126; SAME padding is
    # realized by clipping the band to valid rows.
    bands = np.zeros((2, len(SCALES), KROWS, HT), np.float32)
    for t, row_base in enumerate((0, H - KROWS)):
        for si, r in enumerate(SCALES):
            pb = PADLO[r]
            for o in range(HT):
                h = t * HT + o
                for row in range(h - pb, h - pb + r):
                    k = row - row_base
                    if 0 <= row < H and 0 <= k < KROWS:
                        bands[t, si, k, o] = 1.0
    # Diagonal combine matrices c_r * I, [HT, HT].
    diags = np.zeros((len(SCALES), HT, HT), np.float32)
    for si, r in enumerate(SCALES):
        np.fill_diagonal(diags[si], g * wmap[r])
    params = np.array([b_total, 0.0], np.float32)
    return (bands.astype(np.float16), diags.astype(np.float16), params,
            uni, G.astype(np.float32), Bc.astype(np.float32), K)


def _build_nc():
    if "nc" in _CACHE:
        return _CACHE["nc"]
    import concourse.bass as bass
    import concourse.tile as tile
    from concourse import mybir, bacc, bass_isa
    from contextlib import ExitStack

    f32, f16 = mybir.dt.float32, mybir.dt.float16
    ALU = mybir.AluOpType
    AF = mybir.ActivationFunctionType

    nc = bacc.Bacc("TRN2", target_bir_lowering=False, debug=False,
                   num_devices=N_CORES)
    x_d = nc.dram_tensor("xs", [BPC, H, W, C], f32, kind="ExternalInput").ap()
    bands_d = nc.dram_tensor("bands", [2, 4, KROWS, HT], f16,
                             kind="ExternalInput").ap()
    diags_d = nc.dram_tensor("diags", [4, HT, HT], f16,
                             kind="ExternalInput").ap()
    params_d = nc.dram_tensor("params", [2], f32, kind="ExternalInput").ap()
    out_d = nc.dram_tensor("out", [BPC, H, W, C], f32,
                           kind="ExternalOutput").ap()

    with tile.TileContext(nc) as tc, ExitStack() as ctx:
        P = lambda name, bufs, **kw: ctx.enter_context(
            tc.tile_pool(name=name, bufs=bufs, **kw))
        singles = P("singles", 1)
        xhpool = P("xhpool", 4)
        spool = P("spool", 2)
        lmpool = P("lmpool", 3)
        outpool = P("outpool", 4)
        scal = P("scal", 2)
        ps_A = P("ps_A", 2, space="PSUM")   # scales (16, 8): 2 banks/tile
        ps_B = P("ps_B", 1, space="PSUM")   # scales (4, 2)
        ps_u = P("ps_u", 2, space="PSUM")

        # --- constants to SBUF ---
        bands_sb = [singles.tile([KROWS, 4, HT], f16, tag=f"bands{t}",
                                 name=f"bands_sb{t}") for t in range(2)]
        for t in range(2):
            nc.sync.dma_start(bands_sb[t][:],
                              bands_d[t].transpose([1, 0, 2]))
        diags_sb = singles.tile([HT, 4, HT], f16, tag="diags")
        nc.sync.dma_start(diags_sb[:], diags_d.transpose([1, 0, 2]))
        btot = singles.tile([128, 1], f32, tag="btot")
        nc.sync.dma_start(
            btot[:], bass.AP(tensor=params_d.tensor, offset=0,
                             ap=[[0, 128], [1, 1]]))

        tbase = (0, H - KROWS)   # per-tile DRAM H-row base

        # ------------- emission helpers (software pipeline) -------------
        def emit_load_dma(s):
            st = {"xh": [], "s": s}
            st["mn_strip"] = scal.tile([128, 2], f32, tag="mnst",
                                       name="mnst")
            st["mx_strip"] = scal.tile([128, 2], f32, tag="mxst",
                                       name="mxst")
            nc.vector.memset(st["mn_strip"][:], 3.0e38)
            nc.vector.memset(st["mx_strip"][:], -3.0e38)
            for t in range(2):
                xh = xhpool.tile([KROWS, WP], f16, tag="xh", name="xh")
                # zero margins (write-only: safe on garbage slots)
                nc.vector.memset(xh[:, 0:WM * C], 0.0)
                nc.vector.memset(xh[:, WM * C + FD:WP], 0.0)
                # casting DMA: SWDGE converts f32->f16 in the DMA datapath
                h0 = tbase[t]
                nc.gpsimd.dma_start(xh[:, WM * C:WM * C + FD],
                                    x_d[s, h0:h0 + KROWS, :, :])
                st["xh"].append(xh)
            return st

        def emit_load_reduce(st):
            for t in range(2):
                xh = st["xh"][t]
                # min over the f16 values used downstream (exact in-domain)
                nc.vector.tensor_reduce(
                    out=st["mn_strip"][0:KROWS, t:t + 1],
                    in_=xh[:, WM * C:WM * C + FD],
                    axis=mybir.AxisListType.X, op=ALU.min)
                # subsampled max partials (enters only via tiny EPS')
                xv = xh[:, WM * C:WM * C + FD].rearrange(
                    "p (w c) -> p w c", c=C)[:, ::8, :]
                nc.vector.tensor_reduce(
                    out=st["mx_strip"][0:KROWS, t:t + 1], in_=xv,
                    axis=mybir.AxisListType.XY, op=ALU.max)

        def emit_finalize(st):
            mncol = scal.tile([128, 1], f32, tag="mncol", name="mncol")
            nc.vector.tensor_tensor(mncol[:], st["mn_strip"][:, 0:1],
                                    st["mn_strip"][:, 1:2], op=ALU.min)
            nc.vector.tensor_scalar_mul(mncol[:], mncol[:], -1.0)
            mn_bc = scal.tile([128, 1], f32, tag="mnbc", name="mnbc")
            nc.gpsimd.partition_all_reduce(mn_bc[:], mncol[:], channels=128,
                                           reduce_op=bass_isa.ReduceOp.max)
            nc.vector.tensor_scalar_mul(mn_bc[:], mn_bc[:], -1.0)
            mxcol = scal.tile([128, 1], f32, tag="mxcol", name="mxcol")
            nc.vector.tensor_tensor(mxcol[:], st["mx_strip"][:, 0:1],
                                    st["mx_strip"][:, 1:2], op=ALU.max)
            mx_bc = scal.tile([128, 1], f32, tag="mxbc", name="mxbc")
            nc.gpsimd.partition_all_reduce(mx_bc[:], mxcol[:], channels=128,
                                           reduce_op=bass_isa.ReduceOp.max)
            m2n = scal.tile([128, 1], f32, tag="m2n", name="m2n")
            nc.vector.tensor_scalar_mul(m2n[:], mn_bc[:], 2.0)
            epsp = scal.tile([128, 1], f32, tag="epsp", name="epsp")
            nc.vector.tensor_tensor(epsp[:], mx_bc[:], mn_bc[:],
                                    op=ALU.subtract)
            nc.vector.tensor_scalar(epsp[:], epsp[:], EPS, EPS,
                                    op0=ALU.add, op1=ALU.mult)
            st["m2n"] = m2n
            st["epsb"] = {}
            for gname, r0 in (("A", 16), ("B", 4)):
                e = scal.tile([128, 1], f32, tag=f"epsb{gname}",
                              name=f"epsb{gname}")
                nc.vector.tensor_scalar_mul(e[:], epsp[:], SR[r0])
                st["epsb"][gname] = e
            # fill W margins with mn on DVE ((x*0)+mn; margins were memset 0)
            for t in range(2):
                xh = st["xh"][t]
                for lo, hi in ((0, WM * C), (WM * C + FD, WP)):
                    nc.vector.tensor_scalar(xh[:, lo:hi], xh[:, lo:hi],
                                            0.0, st["m2n"][0:KROWS],
                                            op0=ALU.mult, op1=ALU.add)

        def emit_chain(st, t):
            """W-axis doubling chain for tile t of sample st; margins make
            out-of-range columns exactly zero after the -mn shift."""
            xh = st["xh"][t]
            S = {}
            for r in SCALES:
                lo, hi = CH_RANGE[r]
                S[r] = spool.tile([KROWS, hi - lo], f16, tag=f"S{r}",
                                  name=f"S{r}")
            lo, hi = CH_RANGE[2]
            nc.vector.scalar_tensor_tensor(
                out=S[2][:, :], in0=xh[:, lo:hi], scalar=st["m2n"][0:KROWS],
                in1=xh[:, lo + C:hi + C], op0=ALU.subtract, op1=ALU.add)
            for r, rp, sh in ((4, 2, C), (8, 4, 2 * C), (16, 8, 4 * C)):
                lo, hi = CH_RANGE[r]
                plo = CH_RANGE[rp][0]
                nc.vector.tensor_tensor(
                    S[r][:, :], S[rp][:, lo - sh - plo:hi - sh - plo],
                    S[rp][:, lo + sh - plo:hi + sh - plo], op=ALU.add)
            return S

        prev = None   # pending combine+copyout for the previous chunk

        def flush_prev():
            nonlocal prev
            if prev is None:
                return
            (lmA, lmB), st, t_, c_ = prev
            rhs = {16: lmA[:, 0:NCHUNK], 8: lmA[:, NCHUNK:2 * NCHUNK],
                   4: lmB[:, 0:NCHUNK], 2: lmB[:, NCHUNK:2 * NCHUNK]}
            u = ps_u.tile([HT, NCHUNK], f32, tag="u", name="u")
            for i, r in enumerate(SCALES):
                nc.tensor.matmul(u[:], diags_sb[:, i, :], rhs[r],
                                 start=(i == 0), stop=(i == 3))
            osb = outpool.tile([HT, NCHUNK], f32, tag="osb", name="osb")
            if c_ % 6 == 5:   # a few copyouts on ACT to offload DVE
                nc.scalar.activation(osb[:], u[:], AF.Identity,
                                     bias=btot[0:HT], scale=1.0)
            else:
                nc.vector.tensor_scalar_add(osb[:], u[:], btot[0:HT])
            w0 = c_ * (NCHUNK // C)
            nc.sync.dma_start(
                out_d[st["s"], t_ * HT:(t_ + 1) * HT,
                      w0:w0 + NCHUNK // C, :], osb[:])
            prev = None

        def emit_chunk(st, t, S, c):
            nonlocal prev
            fo = WM * C + c * NCHUNK
            mA = ps_A.tile([HT, 2 * NCHUNK], f32, tag="mA", name="mA")
            mB = ps_B.tile([HT, 2 * NCHUNK], f32, tag="mB", name="mB")
            halves = {16: mA[:, 0:NCHUNK], 8: mA[:, NCHUNK:],
                      4: mB[:, 0:NCHUNK], 2: mB[:, NCHUNK:]}
            for si, r in enumerate(SCALES):
                lo = CH_RANGE[r][0]
                nc.tensor.matmul(halves[r], bands_sb[t][:, si, :],
                                 S[r][:, fo - lo:fo - lo + NCHUNK],
                                 start=True, stop=True)
            flush_prev()
            lmA = lmpool.tile([HT, 2 * NCHUNK], f16, tag="lmA", name="lmA")
            nc.scalar.activation(lmA[:], mA[:], AF.Ln,
                                 bias=st["epsb"]["A"][0:HT], scale=SR[16])
            lmB = lmpool.tile([HT, 2 * NCHUNK], f16, tag="lmB", name="lmB")
            nc.scalar.activation(lmB[:], mB[:], AF.Ln,
                                 bias=st["epsb"]["B"][0:HT], scale=SR[4])
            prev = ((lmA, lmB), st, t, c)

        # ------------------- pipelined emission -------------------
        tiles = [(s, t) for s in range(BPC) for t in range(2)]
        st_by_s = {0: emit_load_dma(0)}
        emit_load_reduce(st_by_s[0])
        emit_finalize(st_by_s[0])
        S_cur = emit_chain(st_by_s[0], 0)
        S_next = None
        for i, (s, t) in enumerate(tiles):
            st = st_by_s[s]
            nxt = tiles[i + 1] if i + 1 < len(tiles) else None
            for c in range(NCH):
                if t == 1 and s + 1 < BPC:
                    if c == 0:
                        st_by_s[s + 1] = emit_load_dma(s + 1)
                    elif c == 4:
                        emit_load_reduce(st_by_s[s + 1])
                    elif c == 6:
                        emit_finalize(st_by_s[s + 1])
                if c == 7 and nxt is not None:
                    S_next = emit_chain(st_by_s[nxt[0]], nxt[1])
                emit_chunk(st, t, S_cur, c)
            S_cur = S_next
        flush_prev()
    nc.compile()
    _CACHE["nc"] = nc
    return nc


def kernel(x, gamma, beta, moving_mean, moving_var):
    from concourse.bass_utils import run_bass_kernel_spmd

    x = np.ascontiguousarray(np.asarray(x, np.float32))
    bands, diags, params, uni, G, Bc, Kc = _host_consts(
        np.asarray(gamma), np.asarray(beta),
        np.asarray(moving_mean), np.asarray(moving_var))
    nc = _build_nc()
    in_maps = [{"xs": x[c * BPC:(c + 1) * BPC], "bands": bands,
                "diags": diags, "params": params} for c in range(N_CORES)]
    res = run_bass_kernel_spmd(nc, in_maps, core_ids=list(range(N_CORES)))
    out = np.concatenate([res.results[c]["out"] for c in range(N_CORES)],
                         axis=0)
    if not uni:
        # general fallback: device ran with g=1,b=0 => out holds raw alphas
        out = out * G[None, None, None, :] + Bc[None, None, None, :]
    return out.astype(np.float32)
